# revision 30
# baseline (speedup 1.0000x reference)
"""LocalSphereAttention Trainium2 kernel.

B=2, N=8192, DIM=256, H=8, HD=32, K=32 neighbors (random idx).

Sharding: 8 cores = 2 batches x 4 query-slices of 2048. Each core
computes K/V/P features for its full batch (replicated compute) into a
fused HBM table (rows [kf 256 | vf 256 | P 32 | pad] f16, 1280B), then
row-gathers the 32 neighbors of each of its 2048 queries with
dma_gather (4 calls/tile on queues 0-3) and does the per-query
attention math on DVE/ACT with PE doing projections + bias-MLP layer 2.

Host-side prep (cheap, layout-only): per-core rotation of the point
cloud so queries are rows 0..2048 (idx remapped mod N), x pre-transposed
and cast to fp16, int16 wrapped gather-index formatting.
"""

import sys
import numpy as np

sys.path.insert(0, "/opt/trn_rl_repo")

import concourse.bass as bass  # noqa: E402
import concourse.bacc as bacc  # noqa: E402
import concourse.tile as tile  # noqa: E402
from concourse import mybir  # noqa: E402
from concourse.masks import make_identity  # noqa: E402

B, N, DIM, H, K = 2, 8192, 256, 8, 32
HD = DIM // H
NC_PER_B = 4
NQ = N // NC_PER_B          # queries per core
P = 128                     # partition tile
T = NQ // P                 # query tiles per core (16)
RW = 640                    # fused f16 row: kf 256 | vf 256 | P 32 | pad 96
VOFF = 256
POFF = 512
XC = DIM // P               # 2 column chunks of contraction dim
F16 = mybir.dt.float16
F32 = mybir.dt.float32
I16 = mybir.dt.int16
MUL = mybir.AluOpType.mult
ADD = mybir.AluOpType.add
SUB = mybir.AluOpType.subtract
MAX = mybir.AluOpType.max

ts = bass.ts


def build_program():
    nc = bacc.Bacc("TRN2", target_bir_lowering=False, debug=False, num_swdge_queues=4)

    xTin = nc.dram_tensor("xTin", [P, XC * N], F16, kind="ExternalInput").ap()
    xyzT = nc.dram_tensor("xyzT", [4, N], F16, kind="ExternalInput").ap()
    idxw = nc.dram_tensor("idxw", [P, T * 256], I16, kind="ExternalInput").ap()
    wq = nc.dram_tensor("wq", [DIM, DIM], F16, kind="ExternalInput").ap()
    wkv = nc.dram_tensor("wkv", [DIM, 2 * DIM], F16, kind="ExternalInput").ap()
    wo = nc.dram_tensor("wo", [DIM, DIM], F16, kind="ExternalInput").ap()
    w1b = nc.dram_tensor("w1b", [4, 32], F16, kind="ExternalInput").ap()
    w2bd = nc.dram_tensor("w2bd", [P, 32], F16, kind="ExternalInput").ap()
    b1r = nc.dram_tensor("b1r", [1, 32], F16, kind="ExternalInput").ap()
    bqr = nc.dram_tensor("bqr", [1, DIM], F16, kind="ExternalInput").ap()
    bor = nc.dram_tensor("bor", [1, DIM], F16, kind="ExternalInput").ap()
    out = nc.dram_tensor("out", [NQ, DIM], F32, kind="ExternalOutput").ap()

    with tile.TileContext(nc) as tc:
        with (
            tc.tile_pool(name="consts", bufs=1) as cpool,
            tc.tile_pool(name="prm", bufs=1) as prmpool,
            tc.tile_pool(name="dram", bufs=1, space="DRAM") as dpool,
        ):
            ident = cpool.tile([P, P], F16)
            make_identity(nc, ident[:])
            ones1 = cpool.tile([1, P], F16)
            nc.gpsimd.memset(ones1[:], 1.0)

            w_sb = {}
            for name, dram in (("wq", wq), ("wo", wo)):
                wt = cpool.tile([P, XC, DIM], F16, tag=f"w_{name}")
                nc.sync.dma_start(wt[:], dram.rearrange("(j p) d -> p j d", p=P))
                w_sb[name] = wt
            wkv_sb = cpool.tile([P, XC, 2 * DIM], F16, tag="w_kv")
            nc.sync.dma_start(wkv_sb[:], wkv.rearrange("(j p) d -> p j d", p=P))
            w1b_sb = cpool.tile([4, 32], F16)
            nc.sync.dma_start(w1b_sb[:], w1b[:])
            w2bd_sb = cpool.tile([P, 32], F16)
            nc.sync.dma_start(w2bd_sb[:], w2bd[:])
            bqr_sb = cpool.tile([1, DIM], F16)
            nc.sync.dma_start(bqr_sb[:], bqr[:])
            bor_sb = cpool.tile([1, DIM], F16)
            nc.sync.dma_start(bor_sb[:], bor[:])
            b1_b = cpool.tile([P, 32], F16)
            nc.sync.dma_start(b1_b[:], b1r.to_broadcast([P, 32]))
            idx_sb = cpool.tile([P, T * 256], I16)
            nc.sync.dma_start(idx_sb[:], idxw[:])

            Pq_b = prmpool.tile([P, T, 32], F16)         # query-side P + b1
            q_all = prmpool.tile([P, T, DIM], F16)       # all projected queries
            kvpd = dpool.tile([N, RW], F16)

            # ---- phase 1+3: load xT, build fused K/V/P table, project Q ----
            with (
                tc.tile_pool(name="xT", bufs=1) as xtpool,
                tc.tile_pool(name="row", bufs=3) as rowpool,
                tc.tile_pool(name="psK", bufs=4, space="PSUM") as psK,
                tc.tile_pool(name="psQ", bufs=2, space="PSUM") as psQ,
                tc.tile_pool(name="psP", bufs=2, space="PSUM") as psP,
            ):
                NCH = 4
                CHN = N // NCH
                xts = []
                xv = xTin.rearrange("p (j n) -> p j n", j=XC)
                for c in range(NCH):
                    xt = xtpool.tile([P, XC, CHN], F16, tag=f"xt{c}")
                    nc.sync.dma_start(xt[:], xv[:, :, c * CHN : (c + 1) * CHN])
                    xts.append(xt)
                xyz_sb = xtpool.tile([4, N], F16, tag="xyz")
                nc.sync.dma_start(xyz_sb[:], xyzT[:])

                TPC = CHN // P  # row tiles per chunk
                for t in range(N // P):
                    xT_t = xts[t // TPC]
                    tt = t % TPC
                    kvps = psK.tile([P, 2 * DIM], F32, tag="kv")
                    for j in range(XC):
                        nc.tensor.matmul(
                            out=kvps[:], lhsT=xT_t[:, j, ts(tt, P)],
                            rhs=wkv_sb[:, j, :],
                            start=(j == 0), stop=(j == XC - 1),
                        )
                    pps = psP.tile([P, 32], F32, tag="pf")
                    nc.tensor.matmul(
                        out=pps[:], lhsT=xyz_sb[0:4, ts(t, P)], rhs=w1b_sb[:],
                        start=True, stop=True,
                    )
                    row = rowpool.tile([P, RW], F16)
                    nc.scalar.copy(out=row[:, 0:POFF], in_=kvps[:])
                    nc.scalar.copy(out=row[:, POFF : POFF + 32], in_=pps[:])
                    if t < T:
                        nc.vector.tensor_tensor(
                            out=Pq_b[:, t, :], in0=pps[:], in1=b1_b[:], op=ADD,
                        )
                        qps = psQ.tile([P, DIM], F32, tag="q")
                        for j in range(XC):
                            nc.tensor.matmul(
                                out=qps[:], lhsT=xT_t[:, j, ts(tt, P)],
                                rhs=w_sb["wq"][:, j, :],
                                start=(j == 0), stop=False,
                            )
                        nc.tensor.matmul(
                            out=qps[:], lhsT=ones1[:], rhs=bqr_sb[:],
                            start=False, stop=True,
                        )
                        nc.vector.tensor_copy(out=q_all[:, t, :], in_=qps[:])
                    nc.sync.dma_start(kvpd[ts(t, P), :], row[:])

            # ---- phase 4: per-query-tile attention ----
            with (
                tc.tile_pool(name="gkv", bufs=2) as gpool,
                tc.tile_pool(name="big", bufs=2) as bigpool,
                tc.tile_pool(name="tre", bufs=1) as trpool,
                tc.tile_pool(name="eb", bufs=2) as ebpool,
                tc.tile_pool(name="sml", bufs=2) as smpool,
                tc.tile_pool(name="psA", bufs=2, space="PSUM") as psA,
                tc.tile_pool(name="psB", bufs=2, space="PSUM") as psB,
                tc.tile_pool(name="psC", bufs=2, space="PSUM") as psC,
            ):
                # stage2 (AV + output projection) of tile t is emitted after
                # stage1 of tile t+1 so prod2(t) — which waits on the long
                # ACT attn-expand — never head-of-line-blocks the DVE FIFO.
                def stage2(t, g, a_b):
                    prod2 = bigpool.tile([P, K * DIM], F16, tag="big")
                    nc.vector.tensor_tensor(
                        out=prod2[:].rearrange("p (k f) -> p k f", f=DIM),
                        in0=g[:, :, VOFF:POFF],
                        in1=a_b[:].rearrange("p (k f) -> p k f", f=DIM),
                        op=MUL,
                    )
                    avps = psA.tile([P, DIM], F32, tag="q")
                    p2v = prod2[:].rearrange("p (k f) -> p k f", f=DIM)
                    for k in range(K):
                        nc.tensor.matmul(
                            out=avps[:], lhsT=ident[:], rhs=p2v[:, k, :],
                            start=(k == 0), stop=(k == K - 1),
                        )
                    oc = smpool.tile([P, DIM], F16, tag="oc")
                    nc.scalar.copy(out=oc[:], in_=avps[:])

                    oT = smpool.tile([P, XC, P], F16, tag="oT")
                    for j in range(XC):
                        ops = psB.tile([P, P], F16, tag="hT")
                        nc.tensor.transpose(ops[:], oc[:, ts(j, P)], ident[:])
                        nc.scalar.copy(out=oT[:, j, :], in_=ops[:])
                    fps = psA.tile([P, DIM], F32, tag="q")
                    for j in range(XC):
                        nc.tensor.matmul(
                            out=fps[:], lhsT=oT[:, j, :], rhs=w_sb["wo"][:, j, :],
                            start=(j == 0), stop=False,
                        )
                    nc.tensor.matmul(
                        out=fps[:], lhsT=ones1[:], rhs=bor_sb[:],
                        start=False, stop=True,
                    )
                    fout = smpool.tile([P, DIM], F32, tag="fout")
                    nc.scalar.copy(out=fout[:], in_=fps[:])
                    nc.sync.dma_start(out[ts(t, P), :], fout[:])

                pending = None
                for t in range(T):
                    g = gpool.tile([P, K, RW], F16)
                    for c in range(4):
                        nc.gpsimd.dma_gather(
                            g[:, 8 * c : 8 * (c + 1), :], kvpd[:],
                            idx_sb[:, t * 256 + 64 * c : t * 256 + 64 * (c + 1)],
                            8 * P, 8 * P, RW, queue_num=c,
                        )

                    # scores: prod = q (bcast over k) * kn ; tree over d
                    prod = bigpool.tile([P, K * DIM], F16, tag="big")
                    nc.vector.tensor_tensor(
                        out=prod[:].rearrange("p (s d) -> p s d", d=DIM),
                        in0=g[:, :, 0:VOFF],
                        in1=q_all[:, t, :].unsqueeze(1).broadcast_to([P, K, DIM]),
                        op=MUL,
                    )
                    scores = smpool.tile([P, K * H], F32, tag="scores")
                    cur = prod[:].rearrange("p (s d) -> p s d", d=HD)
                    w, lvl = HD, 0
                    while w > 2:
                        nxt = trpool.tile(
                            [P, K * H * (w // 2)], F16, tag=f"tr{lvl % 2}"
                        )
                        nv = nxt[:].rearrange("p (s d) -> p s d", d=w // 2)
                        nc.vector.tensor_tensor(
                            out=nv, in0=cur[:, :, 0 : w // 2],
                            in1=cur[:, :, w // 2 : w], op=ADD,
                        )
                        cur, w, lvl = nv, w // 2, lvl + 1
                    nc.vector.tensor_tensor(
                        out=scores[:].rearrange("p (s d) -> p s d", d=1),
                        in0=cur[:, :, 0:1], in1=cur[:, :, 1:2], op=ADD,
                    )

                    # bias MLP: h1 = relu(Pq - Pn) in [p, k, c]
                    h1 = smpool.tile([P, K * 32], F16, tag="h1")
                    nc.vector.tensor_tensor(
                        out=h1[:].rearrange("p (k c) -> p k c", c=32),
                        in0=Pq_b[:, t, :].unsqueeze(1).broadcast_to([P, K, 32]),
                        in1=g[:, :, POFF : POFF + 32],
                        op=SUB,
                    )
                    nc.scalar.activation(
                        h1[:], h1[:], mybir.ActivationFunctionType.Relu
                    )
                    h1T = smpool.tile([P, 8, P], F16, tag="h1T")
                    for j in range(8):
                        hps = psB.tile([P, P], F16, tag="hT")
                        nc.tensor.transpose(hps[:], h1[:, ts(j, P)], ident[:])
                        nc.scalar.copy(out=h1T[:, j, :], in_=hps[:])
                    ob = smpool.tile([32, 8, P], F16, tag="ob")
                    btp = psA.tile([P, K * H], F16, tag="bias")
                    for j in range(8):
                        bps = psC.tile([32, P], F32, tag="l2")
                        nc.tensor.matmul(
                            out=bps[:], lhsT=w2bd_sb[:], rhs=h1T[:, j, :],
                            start=True, stop=True,
                        )
                        nc.scalar.copy(out=ob[:, j, :], in_=bps[:])
                    for j in range(8):
                        nc.tensor.transpose(
                            btp[:, ts(j, 32)], ob[:, j, :], ident[0:32, 0:32]
                        )
                    nc.vector.tensor_tensor(
                        out=scores[:], in0=scores[:], in1=btp[:], op=ADD,
                    )

                    # softmax over k; scores are bounded (|s| < ~10) so skip
                    # the max-subtraction: exp stays well inside f16 range
                    e16 = smpool.tile([P, K * H], F16, tag="e16")
                    nc.scalar.activation(
                        e16[:], scores[:], mybir.ActivationFunctionType.Exp
                    )
                    ssum = smpool.tile([P, H], F32, tag="ssum")
                    nc.vector.tensor_reduce(
                        out=ssum[:], in_=e16[:].rearrange("p (k h) -> p h k", h=H),
                        axis=mybir.AxisListType.X, op=ADD,
                    )
                    rec = smpool.tile([P, H], F32, tag="rec")
                    nc.vector.reciprocal(rec[:], ssum[:])
                    attn = smpool.tile([P, K * H], F16, tag="attn")
                    nc.vector.tensor_tensor(
                        out=attn[:].rearrange("p (k h) -> p k h", h=H),
                        in0=e16[:].rearrange("p (k h) -> p k h", h=H),
                        in1=rec[:].unsqueeze(1).broadcast_to([P, K, H]),
                        op=MUL,
                    )
                    # normalized attn expanded across d on ACT (keeps AV mul 2x)
                    a_b = ebpool.tile([P, K * DIM], F16)
                    nc.scalar.activation(
                        a_b[:].rearrange("p (k h d) -> p k h d", h=H, d=HD),
                        attn[:]
                        .rearrange("p (k h) -> p k h", h=H)
                        .unsqueeze(3)
                        .broadcast_to([P, K, H, HD]),
                        mybir.ActivationFunctionType.Copy,
                    )

                    if pending is not None:
                        stage2(*pending)
                    pending = (t, g, a_b)
                stage2(*pending)

    nc.compile()
    return nc


_NC = None


def _get_nc():
    global _NC
    if _NC is None:
        _NC = build_program()
    return _NC


def make_in_maps(inputs):
    x = np.asarray(inputs["x"], np.float32)
    xyz = np.asarray(inputs["xyz"], np.float32)
    idx = np.asarray(inputs["idx"], np.int32)
    f16 = lambda a: np.asarray(a, np.float32).astype(np.float16)

    s = 1.0 / np.sqrt(HD)
    wq = f16(np.asarray(inputs["Wq"], np.float32) * s)
    bq = f16(np.asarray(inputs["bq"], np.float32) * s).reshape(1, DIM)
    wkv = np.concatenate(
        [np.asarray(inputs["Wk"], np.float32), np.asarray(inputs["Wv"], np.float32)],
        axis=1,
    ).astype(np.float16)
    wo = f16(inputs["Wo"])
    w1b = np.concatenate(
        [np.asarray(inputs["W1"], np.float32), np.zeros((1, 32), np.float32)],
        axis=0,
    ).astype(np.float16)  # [4, 32] (no bias; b1 applied query-side only)
    b1rr = f16(inputs["b1"]).reshape(1, 32)
    # W2 block-diag for l2-on-PE: W2bd[k'*32+c, k''*8+h] = (k'==k'') W2[c,h]
    W2 = np.asarray(inputs["W2"], np.float32)  # [32, 8]
    w2bd = np.zeros((P, 32), np.float32)
    for kp in range(4):
        w2bd[kp * 32 : (kp + 1) * 32, kp * 8 : (kp + 1) * 8] = W2
    w2bd = w2bd.astype(np.float16)
    # bo' = bo + bv @ Wo (bv folds through attention-sum=1); b2, bk cancel
    bo = (
        np.asarray(inputs["bo"], np.float32)
        + np.asarray(inputs["bv"], np.float32) @ np.asarray(inputs["Wo"], np.float32)
    ).reshape(1, DIM).astype(np.float16)

    in_maps = []
    for c in range(8):
        b = c // NC_PER_B
        qs = (c % NC_PER_B) * NQ
        xr = np.roll(x[b], -qs, axis=0).astype(np.float16)
        # xT[p, j, n] = x[n, j*128+p]
        xT = (
            np.ascontiguousarray(xr.T.reshape(XC, P, N).transpose(1, 0, 2))
            .reshape(P, XC * N)
        )
        xyzr = np.roll(xyz[b], -qs, axis=0)
        xyzTa = np.concatenate(
            [np.ascontiguousarray(xyzr.T), np.ones((1, N), np.float32)], axis=0
        ).astype(np.float16)
        ir = ((idx[b, qs : qs + NQ, :].astype(np.int64) - qs) % N).astype(np.int16)
        # per tile: order i = k*128 + n_local, wrapped [16, 256], replicate x8
        blocks = []
        for t in range(T):
            A = ir[t * P : (t + 1) * P, :].T.reshape(-1)  # [K*P], i = k*128+n
            blocks.append(A.reshape(256, 16).T)  # [16, 256]
        iw16 = np.concatenate(blocks, axis=1)  # [16, T*256]
        iw = np.tile(iw16, (8, 1)).astype(np.int16)  # [128, T*256]
        in_maps.append(
            {
                "xTin": np.ascontiguousarray(xT),
                "xyzT": np.ascontiguousarray(xyzTa),
                "idxw": np.ascontiguousarray(iw),
                "wq": wq, "wkv": wkv, "wo": wo,
                "w1b": w1b, "w2bd": w2bd, "bqr": bq, "bor": bo, "b1r": b1rr,
            }
        )
    return in_maps


def kernel(**inputs) -> np.ndarray:
    from concourse import bass_utils

    nc = _get_nc()
    in_maps = make_in_maps(inputs)
    res = bass_utils.run_bass_kernel_spmd(nc, in_maps, core_ids=list(range(8)))
    outp = np.zeros((B, N, DIM), np.float32)
    for c in range(8):
        b = c // NC_PER_B
        qs = (c % NC_PER_B) * NQ
        outp[b, qs : qs + NQ, :] = res.results[c]["out"]
    return outp


# revision 31
# speedup vs baseline: 1.0454x; 1.0454x over previous
"""LocalSphereAttention Trainium2 kernel.

B=2, N=8192, DIM=256, H=8, HD=32, K=32 neighbors (random idx).

Sharding: 8 cores = 2 batches x 4 query-slices of 2048. Each core
computes K/V/P features for its full batch (replicated compute) into a
fused HBM table (rows [kf 256 | vf 256 | P 32 | pad] f16, 1280B), then
row-gathers the 32 neighbors of each of its 2048 queries with
dma_gather (4 calls/tile on queues 0-3) and does the per-query
attention math on DVE/ACT with PE doing projections + bias-MLP layer 2.

Host-side prep (cheap, layout-only): per-core rotation of the point
cloud so queries are rows 0..2048 (idx remapped mod N), x pre-transposed
and cast to fp16, int16 wrapped gather-index formatting.
"""

import sys
import numpy as np

sys.path.insert(0, "/opt/trn_rl_repo")

import concourse.bass as bass  # noqa: E402
import concourse.bacc as bacc  # noqa: E402
import concourse.tile as tile  # noqa: E402
from concourse import mybir  # noqa: E402
from concourse.masks import make_identity  # noqa: E402

B, N, DIM, H, K = 2, 8192, 256, 8, 32
HD = DIM // H
NC_PER_B = 4
NQ = N // NC_PER_B          # queries per core
P = 128                     # partition tile
T = NQ // P                 # query tiles per core (16)
RW = 640                    # fused f16 row: kf 256 | vf 256 | P 32 | pad 96
VOFF = 256
POFF = 512
XC = DIM // P               # 2 column chunks of contraction dim
F16 = mybir.dt.float16
F32 = mybir.dt.float32
I16 = mybir.dt.int16
MUL = mybir.AluOpType.mult
ADD = mybir.AluOpType.add
SUB = mybir.AluOpType.subtract
MAX = mybir.AluOpType.max

ts = bass.ts


def build_program():
    nc = bacc.Bacc("TRN2", target_bir_lowering=False, debug=False, num_swdge_queues=4)

    xTin = nc.dram_tensor("xTin", [P, XC * N], F16, kind="ExternalInput").ap()
    xyzT = nc.dram_tensor("xyzT", [4, N], F16, kind="ExternalInput").ap()
    idxw = nc.dram_tensor("idxw", [P, T * 256], I16, kind="ExternalInput").ap()
    wq = nc.dram_tensor("wq", [DIM, DIM], F16, kind="ExternalInput").ap()
    wkv = nc.dram_tensor("wkv", [DIM, 2 * DIM], F16, kind="ExternalInput").ap()
    wo = nc.dram_tensor("wo", [DIM, DIM], F16, kind="ExternalInput").ap()
    w1b = nc.dram_tensor("w1b", [4, 32], F16, kind="ExternalInput").ap()
    w2bd = nc.dram_tensor("w2bd", [P, 32], F16, kind="ExternalInput").ap()
    b1r = nc.dram_tensor("b1r", [1, 32], F16, kind="ExternalInput").ap()
    bqr = nc.dram_tensor("bqr", [1, DIM], F16, kind="ExternalInput").ap()
    bor = nc.dram_tensor("bor", [1, DIM], F16, kind="ExternalInput").ap()
    out = nc.dram_tensor("out", [NQ, DIM], F32, kind="ExternalOutput").ap()

    with tile.TileContext(nc) as tc:
        with (
            tc.tile_pool(name="consts", bufs=1) as cpool,
            tc.tile_pool(name="prm", bufs=1) as prmpool,
            tc.tile_pool(name="dram", bufs=1, space="DRAM") as dpool,
        ):
            ident = cpool.tile([P, P], F16)
            make_identity(nc, ident[:])
            ones1 = cpool.tile([1, P], F16)
            nc.gpsimd.memset(ones1[:], 1.0)

            w_sb = {}
            for name, dram in (("wq", wq), ("wo", wo)):
                wt = cpool.tile([P, XC, DIM], F16, tag=f"w_{name}")
                nc.sync.dma_start(wt[:], dram.rearrange("(j p) d -> p j d", p=P))
                w_sb[name] = wt
            wkv_sb = cpool.tile([P, XC, 2 * DIM], F16, tag="w_kv")
            nc.sync.dma_start(wkv_sb[:], wkv.rearrange("(j p) d -> p j d", p=P))
            w1b_sb = cpool.tile([4, 32], F16)
            nc.sync.dma_start(w1b_sb[:], w1b[:])
            w2bd_sb = cpool.tile([P, 32], F16)
            nc.sync.dma_start(w2bd_sb[:], w2bd[:])
            bqr_sb = cpool.tile([1, DIM], F16)
            nc.sync.dma_start(bqr_sb[:], bqr[:])
            bor_sb = cpool.tile([1, DIM], F16)
            nc.sync.dma_start(bor_sb[:], bor[:])
            b1_b = cpool.tile([P, 32], F16)
            nc.sync.dma_start(b1_b[:], b1r.to_broadcast([P, 32]))
            idx_sb = cpool.tile([P, T * 256], I16)
            nc.sync.dma_start(idx_sb[:], idxw[:])

            Pq_b = prmpool.tile([P, T, 32], F16)         # query-side P + b1
            q_all = prmpool.tile([P, T, DIM], F16)       # all projected queries
            kvpd = dpool.tile([N, RW], F16)

            # ---- phase 1+3: load xT, build fused K/V/P table, project Q ----
            with (
                tc.tile_pool(name="xT", bufs=1) as xtpool,
                tc.tile_pool(name="row", bufs=3) as rowpool,
                tc.tile_pool(name="psK", bufs=4, space="PSUM") as psK,
                tc.tile_pool(name="psQ", bufs=2, space="PSUM") as psQ,
                tc.tile_pool(name="psP", bufs=2, space="PSUM") as psP,
            ):
                NCH = 4
                CHN = N // NCH
                xts = []
                xv = xTin.rearrange("p (j n) -> p j n", j=XC)
                for c in range(NCH):
                    xt = xtpool.tile([P, XC, CHN], F16, tag=f"xt{c}")
                    nc.sync.dma_start(xt[:], xv[:, :, c * CHN : (c + 1) * CHN])
                    xts.append(xt)
                xyz_sb = xtpool.tile([4, N], F16, tag="xyz")
                nc.sync.dma_start(xyz_sb[:], xyzT[:])

                TPC = CHN // P  # row tiles per chunk
                for t in range(N // P):
                    xT_t = xts[t // TPC]
                    tt = t % TPC
                    kvps = psK.tile([P, 2 * DIM], F32, tag="kv")
                    for j in range(XC):
                        nc.tensor.matmul(
                            out=kvps[:], lhsT=xT_t[:, j, ts(tt, P)],
                            rhs=wkv_sb[:, j, :],
                            start=(j == 0), stop=(j == XC - 1),
                        )
                    pps = psP.tile([P, 32], F32, tag="pf")
                    nc.tensor.matmul(
                        out=pps[:], lhsT=xyz_sb[0:4, ts(t, P)], rhs=w1b_sb[:],
                        start=True, stop=True,
                    )
                    row = rowpool.tile([P, RW], F16)
                    nc.scalar.copy(out=row[:, 0:POFF], in_=kvps[:])
                    nc.scalar.copy(out=row[:, POFF : POFF + 32], in_=pps[:])
                    if t < T:
                        nc.vector.tensor_tensor(
                            out=Pq_b[:, t, :], in0=pps[:], in1=b1_b[:], op=ADD,
                        )
                        qps = psQ.tile([P, DIM], F32, tag="q")
                        for j in range(XC):
                            nc.tensor.matmul(
                                out=qps[:], lhsT=xT_t[:, j, ts(tt, P)],
                                rhs=w_sb["wq"][:, j, :],
                                start=(j == 0), stop=False,
                            )
                        nc.tensor.matmul(
                            out=qps[:], lhsT=ones1[:], rhs=bqr_sb[:],
                            start=False, stop=True,
                        )
                        nc.vector.tensor_copy(out=q_all[:, t, :], in_=qps[:])
                    nc.sync.dma_start(kvpd[ts(t, P), :], row[:])

            # ---- phase 4: per-query-tile attention ----
            with (
                tc.tile_pool(name="gkv", bufs=2) as gpool,
                tc.tile_pool(name="big", bufs=2) as bigpool,
                tc.tile_pool(name="tre", bufs=1) as trpool,
                tc.tile_pool(name="eb", bufs=2) as ebpool,
                tc.tile_pool(name="sml", bufs=2) as smpool,
                tc.tile_pool(name="psA", bufs=2, space="PSUM") as psA,
                tc.tile_pool(name="psB", bufs=2, space="PSUM") as psB,
                tc.tile_pool(name="psC", bufs=2, space="PSUM") as psC,
            ):
                for t in range(T):
                    g = gpool.tile([P, K, RW], F16)
                    for c in range(4):
                        nc.gpsimd.dma_gather(
                            g[:, 8 * c : 8 * (c + 1), :], kvpd[:],
                            idx_sb[:, t * 256 + 64 * c : t * 256 + 64 * (c + 1)],
                            8 * P, 8 * P, RW, queue_num=c,
                        )

                    # scores: prod = q (bcast over k) * kn ; tree over d
                    prod = bigpool.tile([P, K * DIM], F16, tag="big")
                    nc.vector.tensor_tensor(
                        out=prod[:].rearrange("p (s d) -> p s d", d=DIM),
                        in0=g[:, :, 0:VOFF],
                        in1=q_all[:, t, :].unsqueeze(1).broadcast_to([P, K, DIM]),
                        op=MUL,
                    )
                    scores = smpool.tile([P, K * H], F32, tag="scores")
                    cur = prod[:].rearrange("p (s d) -> p s d", d=HD)
                    w, lvl = HD, 0
                    while w > 2:
                        nxt = trpool.tile(
                            [P, K * H * (w // 2)], F16, tag=f"tr{lvl % 2}"
                        )
                        nv = nxt[:].rearrange("p (s d) -> p s d", d=w // 2)
                        nc.vector.tensor_tensor(
                            out=nv, in0=cur[:, :, 0 : w // 2],
                            in1=cur[:, :, w // 2 : w], op=ADD,
                        )
                        cur, w, lvl = nv, w // 2, lvl + 1
                    nc.vector.tensor_tensor(
                        out=scores[:].rearrange("p (s d) -> p s d", d=1),
                        in0=cur[:, :, 0:1], in1=cur[:, :, 1:2], op=ADD,
                    )

                    # bias MLP: h1 = relu(Pq - Pn) in [p, k, c]
                    h1 = smpool.tile([P, K * 32], F16, tag="h1")
                    nc.vector.tensor_tensor(
                        out=h1[:].rearrange("p (k c) -> p k c", c=32),
                        in0=Pq_b[:, t, :].unsqueeze(1).broadcast_to([P, K, 32]),
                        in1=g[:, :, POFF : POFF + 32],
                        op=SUB,
                    )
                    nc.scalar.activation(
                        h1[:], h1[:], mybir.ActivationFunctionType.Relu
                    )
                    h1T = smpool.tile([P, 8, P], F16, tag="h1T")
                    for j in range(8):
                        hps = psB.tile([P, P], F16, tag="hT")
                        nc.tensor.transpose(hps[:], h1[:, ts(j, P)], ident[:])
                        nc.scalar.copy(out=h1T[:, j, :], in_=hps[:])
                    ob = smpool.tile([32, 8, P], F16, tag="ob")
                    btp = psA.tile([P, K * H], F16, tag="bias")
                    for j in range(8):
                        bps = psC.tile([32, P], F32, tag="l2")
                        nc.tensor.matmul(
                            out=bps[:], lhsT=w2bd_sb[:], rhs=h1T[:, j, :],
                            start=True, stop=True,
                        )
                        nc.scalar.copy(out=ob[:, j, :], in_=bps[:])
                    for j in range(8):
                        nc.tensor.transpose(
                            btp[:, ts(j, 32)], ob[:, j, :], ident[0:32, 0:32]
                        )
                    nc.vector.tensor_tensor(
                        out=scores[:], in0=scores[:], in1=btp[:], op=ADD,
                    )

                    # softmax over k; scores are bounded (|s| < ~10) so skip
                    # the max-subtraction: exp stays well inside f16 range
                    e16 = smpool.tile([P, K * H], F16, tag="e16")
                    nc.scalar.activation(
                        e16[:], scores[:], mybir.ActivationFunctionType.Exp
                    )
                    ssum = smpool.tile([P, H], F32, tag="ssum")
                    nc.vector.tensor_reduce(
                        out=ssum[:], in_=e16[:].rearrange("p (k h) -> p h k", h=H),
                        axis=mybir.AxisListType.X, op=ADD,
                    )
                    rec = smpool.tile([P, H], F32, tag="rec")
                    nc.vector.reciprocal(rec[:], ssum[:])
                    attn = smpool.tile([P, K * H], F16, tag="attn")
                    nc.vector.tensor_tensor(
                        out=attn[:].rearrange("p (k h) -> p k h", h=H),
                        in0=e16[:].rearrange("p (k h) -> p k h", h=H),
                        in1=rec[:].unsqueeze(1).broadcast_to([P, K, H]),
                        op=MUL,
                    )
                    # normalized attn expanded across d on ACT (keeps AV mul 2x)
                    a_b = ebpool.tile([P, K * DIM], F16)
                    nc.scalar.activation(
                        a_b[:].rearrange("p (k h d) -> p k h d", h=H, d=HD),
                        attn[:]
                        .rearrange("p (k h) -> p k h", h=H)
                        .unsqueeze(3)
                        .broadcast_to([P, K, H, HD]),
                        mybir.ActivationFunctionType.Copy,
                    )

                    # AV: prod2 = a_b * vn (both dense), k-sum on PE via identity
                    prod2 = bigpool.tile([P, K * DIM], F16, tag="big")
                    nc.vector.tensor_tensor(
                        out=prod2[:].rearrange("p (k f) -> p k f", f=DIM),
                        in0=g[:, :, VOFF:POFF],
                        in1=a_b[:].rearrange("p (k f) -> p k f", f=DIM),
                        op=MUL,
                    )
                    avps = psA.tile([P, DIM], F32, tag="q")
                    p2v = prod2[:].rearrange("p (k f) -> p k f", f=DIM)
                    for k in range(K):
                        nc.tensor.matmul(
                            out=avps[:], lhsT=ident[:], rhs=p2v[:, k, :],
                            start=(k == 0), stop=(k == K - 1),
                        )
                    oc = smpool.tile([P, DIM], F16, tag="oc")
                    nc.scalar.copy(out=oc[:], in_=avps[:])

                    # output projection (+bo via ones row)
                    oT = smpool.tile([P, XC, P], F16, tag="oT")
                    for j in range(XC):
                        ops = psB.tile([P, P], F16, tag="hT")
                        nc.tensor.transpose(ops[:], oc[:, ts(j, P)], ident[:])
                        nc.scalar.copy(out=oT[:, j, :], in_=ops[:])
                    fps = psA.tile([P, DIM], F32, tag="q")
                    for j in range(XC):
                        nc.tensor.matmul(
                            out=fps[:], lhsT=oT[:, j, :], rhs=w_sb["wo"][:, j, :],
                            start=(j == 0), stop=False,
                        )
                    nc.tensor.matmul(
                        out=fps[:], lhsT=ones1[:], rhs=bor_sb[:],
                        start=False, stop=True,
                    )
                    fout = smpool.tile([P, DIM], F32, tag="fout")
                    nc.scalar.copy(out=fout[:], in_=fps[:])
                    nc.sync.dma_start(out[ts(t, P), :], fout[:])

    nc.compile()
    return nc


_NC = None


def _get_nc():
    global _NC
    if _NC is None:
        _NC = build_program()
    return _NC


def make_in_maps(inputs):
    x = np.asarray(inputs["x"], np.float32)
    xyz = np.asarray(inputs["xyz"], np.float32)
    idx = np.asarray(inputs["idx"], np.int32)
    f16 = lambda a: np.asarray(a, np.float32).astype(np.float16)

    s = 1.0 / np.sqrt(HD)
    wq = f16(np.asarray(inputs["Wq"], np.float32) * s)
    bq = f16(np.asarray(inputs["bq"], np.float32) * s).reshape(1, DIM)
    wkv = np.concatenate(
        [np.asarray(inputs["Wk"], np.float32), np.asarray(inputs["Wv"], np.float32)],
        axis=1,
    ).astype(np.float16)
    wo = f16(inputs["Wo"])
    w1b = np.concatenate(
        [np.asarray(inputs["W1"], np.float32), np.zeros((1, 32), np.float32)],
        axis=0,
    ).astype(np.float16)  # [4, 32] (no bias; b1 applied query-side only)
    b1rr = f16(inputs["b1"]).reshape(1, 32)
    # W2 block-diag for l2-on-PE: W2bd[k'*32+c, k''*8+h] = (k'==k'') W2[c,h]
    W2 = np.asarray(inputs["W2"], np.float32)  # [32, 8]
    w2bd = np.zeros((P, 32), np.float32)
    for kp in range(4):
        w2bd[kp * 32 : (kp + 1) * 32, kp * 8 : (kp + 1) * 8] = W2
    w2bd = w2bd.astype(np.float16)
    # bo' = bo + bv @ Wo (bv folds through attention-sum=1); b2, bk cancel
    bo = (
        np.asarray(inputs["bo"], np.float32)
        + np.asarray(inputs["bv"], np.float32) @ np.asarray(inputs["Wo"], np.float32)
    ).reshape(1, DIM).astype(np.float16)

    in_maps = []
    for c in range(8):
        b = c // NC_PER_B
        qs = (c % NC_PER_B) * NQ
        xr = np.roll(x[b], -qs, axis=0).astype(np.float16)
        # xT[p, j, n] = x[n, j*128+p]
        xT = (
            np.ascontiguousarray(xr.T.reshape(XC, P, N).transpose(1, 0, 2))
            .reshape(P, XC * N)
        )
        xyzr = np.roll(xyz[b], -qs, axis=0)
        xyzTa = np.concatenate(
            [np.ascontiguousarray(xyzr.T), np.ones((1, N), np.float32)], axis=0
        ).astype(np.float16)
        ir = ((idx[b, qs : qs + NQ, :].astype(np.int64) - qs) % N).astype(np.int16)
        # per tile: order i = k*128 + n_local, wrapped [16, 256], replicate x8
        blocks = []
        for t in range(T):
            A = ir[t * P : (t + 1) * P, :].T.reshape(-1)  # [K*P], i = k*128+n
            blocks.append(A.reshape(256, 16).T)  # [16, 256]
        iw16 = np.concatenate(blocks, axis=1)  # [16, T*256]
        iw = np.tile(iw16, (8, 1)).astype(np.int16)  # [128, T*256]
        in_maps.append(
            {
                "xTin": np.ascontiguousarray(xT),
                "xyzT": np.ascontiguousarray(xyzTa),
                "idxw": np.ascontiguousarray(iw),
                "wq": wq, "wkv": wkv, "wo": wo,
                "w1b": w1b, "w2bd": w2bd, "bqr": bq, "bor": bo, "b1r": b1rr,
            }
        )
    return in_maps


def kernel(**inputs) -> np.ndarray:
    from concourse import bass_utils

    nc = _get_nc()
    in_maps = make_in_maps(inputs)
    res = bass_utils.run_bass_kernel_spmd(nc, in_maps, core_ids=list(range(8)))
    outp = np.zeros((B, N, DIM), np.float32)
    for c in range(8):
        b = c // NC_PER_B
        qs = (c % NC_PER_B) * NQ
        outp[b, qs : qs + NQ, :] = res.results[c]["out"]
    return outp
